# revision 21
# baseline (speedup 1.0000x reference)
"""Multi-head attention kernel for 8 Trainium2 NeuronCores.

Problem: B=4, T=2048, DIM=1024, 16 heads, head_dim=64, additive mask.
  q,k,v = x@W{q,k,v}.T ; attn = softmax(q k^T/8 + mask) ; out = (attn v)@Wo.T

Sharding (no collectives): core i handles batch b=i//2 and half the queries
(1024 rows, interleaved 128-row blocks by parity p=i%2 so the causal work
profile is identical on every core -> one SPMD graph). K/V projections are
computed for the full 2048 keys on both cores of a pair (duplicated compute,
far cheaper than any 2-rank collective on this chip).

On-chip math: exp(s+m) = exp(s)*exp(m); host precomputes exp(mask).
Softmax denominator comes free from a ones-column appended to V.
All matmul operands bf16; PSUM accumulation and softmax math fp32.
"""

import sys
import numpy as np

sys.path.insert(0, "/opt/trn_rl_repo")

import ml_dtypes  # noqa: E402
from contextlib import ExitStack  # noqa: E402
from concourse import bass, bacc, tile  # noqa: E402
from concourse.bass_utils import run_bass_kernel_spmd  # noqa: E402

mybir = bass.mybir

B, T, DIM, H, HD = 4, 2048, 1024, 16, 64
TQ = 1024          # queries per core
NDC = DIM // 128   # 8 contraction chunks
NTC = T // 128     # 16 key chunks
BF16 = mybir.dt.bfloat16
F32 = mybir.dt.float32


def _causal_sched():
    """Emask-mult schedule for the causal variant.

    Group a in {0,1} covers logical query blocks 4a..4a+3 (columns a*512..).
    Active key chunks for group a: 0..8a+7.  Logical block g=4a+bb holds
    queries whose diagonal chunk is 2g+p (parity p in {0,1}); chunks past
    2g+1 are fully masked for both parities and are skipped entirely (the
    masked blocks form a contiguous prefix of each 512-col group).  Exactly
    one block per chunk needs the emask multiply: bb = (c-8a)//2.
    Returns list of (a, c, bb) in canonical order.
    """
    sched = []
    for a in (0, 1):
        for c in range(8 * a + 8):
            sched.append((a, c, (c - 8 * a) // 2))
    return sched


CAUSAL_SCHED = _causal_sched()


def _build(variant, reps=1):
    """variant: 'causal' | 'general' | 'nomask'."""
    nc = bacc.Bacc("TRN2", target_bir_lowering=False, debug=False, num_devices=8)

    xT = nc.dram_tensor("xT", [DIM, T], BF16, kind="ExternalInput").ap()
    xqT = nc.dram_tensor("xqT", [DIM, TQ], BF16, kind="ExternalInput").ap()
    wqT = nc.dram_tensor("wqT", [DIM, DIM], BF16, kind="ExternalInput").ap()
    wkT = nc.dram_tensor("wkT", [DIM, DIM], BF16, kind="ExternalInput").ap()
    wvT = nc.dram_tensor("wvT", [DIM, DIM], BF16, kind="ExternalInput").ap()
    woT = nc.dram_tensor("woT", [DIM, DIM], BF16, kind="ExternalInput").ap()
    if variant == "causal":
        em = nc.dram_tensor(
            "em", [128, len(CAUSAL_SCHED) * 128], BF16, kind="ExternalInput"
        ).ap()
    elif variant == "general":
        em = nc.dram_tensor("em", [T, TQ], BF16, kind="ExternalInput").ap()
    else:
        em = None
    out = nc.dram_tensor("out", [TQ, DIM], F32, kind="ExternalOutput").ap()

    with tile.TileContext(nc) as tc:
      for _rep in range(reps):
        ctx = ExitStack()
        ctx.__enter__()
        Exp = mybir.ActivationFunctionType.Exp
        mult = mybir.AluOpType.mult

        # ---- resident SBUF pools -------------------------------------------
        qt_p = ctx.enter_context(tc.tile_pool(name="qt", bufs=1))
        kt_p = ctx.enter_context(tc.tile_pool(name="kt", bufs=1))
        va_p = ctx.enter_context(tc.tile_pool(name="va", bufs=1))
        misc_p = ctx.enter_context(tc.tile_pool(name="misc", bufs=1))
        # PSUM pools: psS 2x[128,1024] (4 banks), psO 1x[128,1024] (2 banks),
        # psB 1x[64,512] (1 bank), psF 1x[128,512] (1 bank) = 8 banks.
        psS = ctx.enter_context(tc.tile_pool(name="psS", bufs=2, space="PSUM"))
        psO = ctx.enter_context(tc.tile_pool(name="psO", bufs=1, space="PSUM"))
        psB = ctx.enter_context(tc.tile_pool(name="psB", bufs=1, space="PSUM"))
        psF = ctx.enter_context(tc.tile_pool(name="psF", bufs=1, space="PSUM"))

        qt_sb = [qt_p.tile([128, TQ], BF16, tag=f"qt{i}", name=f"qt{i}") for i in range(NDC)]
        kt_sb = [kt_p.tile([128, T], BF16, tag=f"kt{i}", name=f"kt{i}") for i in range(NDC)]
        # V interleaved with ones column: per key chunk [128, 16*65]
        va_sb = [va_p.tile([128, H * 65], BF16, tag=f"va{i}", name=f"va{i}") for i in range(NTC)]
        # ones [1,64] for the reciprocal broadcast matmul
        ones_sb = misc_p.tile([1, 64], F32, tag="ones", name="ones")
        nc.vector.memset(ones_sb[:], 1.0)

        # ---- phase 1: projections ------------------------------------------
        with tc.tile_pool(name="xin", bufs=1) as x_p, tc.tile_pool(
            name="win", bufs=1
        ) as w_p:
            xt_sb = [x_p.tile([128, T], BF16, tag=f"xt{i}", name=f"xt{i}") for i in range(NDC)]
            xq_sb = [x_p.tile([128, TQ], BF16, tag=f"xq{i}", name=f"xq{i}") for i in range(NDC)]
            wq_sb = [w_p.tile([128, DIM], BF16, tag=f"wq{i}", name=f"wq{i}") for i in range(NDC)]
            wk_sb = [w_p.tile([128, DIM], BF16, tag=f"wk{i}", name=f"wk{i}") for i in range(NDC)]
            wv_sb = [w_p.tile([128, DIM], BF16, tag=f"wv{i}", name=f"wv{i}") for i in range(NDC)]
            # issue order matches consumption: V-proj inputs first
            for i in range(NDC):
                s = slice(i * 128, (i + 1) * 128)
                nc.sync.dma_start(wv_sb[i][:], wvT[s, :])
                nc.sync.dma_start(xt_sb[i][:], xT[s, :])
            for i in range(NDC):
                s = slice(i * 128, (i + 1) * 128)
                nc.sync.dma_start(wq_sb[i][:], wqT[s, :])
                nc.sync.dma_start(xq_sb[i][:], xqT[s, :])
            for i in range(NDC):
                s = slice(i * 128, (i + 1) * 128)
                nc.sync.dma_start(wk_sb[i][:], wkT[s, :])

            # ones column of V at position 64 for every head
            for c in range(NTC):
                v4 = va_sb[c][:].rearrange("p (q t x) -> p q t x", q=8, t=2)
                nc.vector.memset(v4[:, :, 0:2, 64:65], 1.0)

            # V projection: V[tk,o] = sum_d xT[d,tk] * wvT[d,o]
            for c in range(NTC):
                csl = slice(c * 128, (c + 1) * 128)
                ps = psS.tile([128, 1024], F32, tag="s", name="v_ps")
                for n in range(2):
                    nsl = slice(n * 512, (n + 1) * 512)
                    for dc in range(NDC):
                        nc.tensor.matmul(
                            ps[:, nsl],
                            xt_sb[dc][:, csl],
                            wv_sb[dc][:, nsl],
                            start=(dc == 0),
                            stop=(dc == NDC - 1),
                        )
                # scatter 16 heads into the 65-stride layout
                v4 = va_sb[c][:].rearrange("p (q t x) -> p q t x", q=8, t=2)
                s4 = ps[:].rearrange("p (q t x) -> p q t x", q=8, t=2)
                nc.vector.tensor_copy(v4[:, :, 0:2, 0:64], s4[:, :, 0:2, :])

            # Q/K projections: out QT[o,t] = sum_d wqT[d,o] * xqT[d,t]
            for oc in range(NDC):
                osl = slice(oc * 128, (oc + 1) * 128)
                ps = psS.tile([128, 1024], F32, tag="s", name="q_ps")
                for n in range(2):
                    nsl = slice(n * 512, (n + 1) * 512)
                    for dc in range(NDC):
                        nc.tensor.matmul(
                            ps[:, nsl],
                            wq_sb[dc][:, osl],
                            xq_sb[dc][:, nsl],
                            start=(dc == 0),
                            stop=(dc == NDC - 1),
                        )
                nc.scalar.copy(qt_sb[oc][:], ps[:])
                for m in range(2):
                    ps = psS.tile([128, 1024], F32, tag="s", name="k_ps")
                    for n in range(2):
                        nsl = slice((2 * m + n) * 512, (2 * m + n + 1) * 512)
                        psl = slice(n * 512, (n + 1) * 512)
                        for dc in range(NDC):
                            nc.tensor.matmul(
                                ps[:, psl],
                                wk_sb[dc][:, osl],
                                xt_sb[dc][:, nsl],
                                start=(dc == 0),
                                stop=(dc == NDC - 1),
                            )
                    nc.scalar.copy(
                        kt_sb[oc][:, m * 1024 : (m + 1) * 1024], ps[:]
                    )


        # ---- phase 2 pools (reuse the projection input space) --------------
        em_p = ctx.enter_context(tc.tile_pool(name="em", bufs=1))
        wo_p = ctx.enter_context(tc.tile_pool(name="wo", bufs=1))
        otn_p = ctx.enter_context(tc.tile_pool(name="otn", bufs=1))
        deep = variant == "causal"
        p_p = ctx.enter_context(tc.tile_pool(name="pp", bufs=6 if deep else 4))
        osb_p = ctx.enter_context(tc.tile_pool(name="osb", bufs=3 if deep else 2))
        nrm_p = ctx.enter_context(tc.tile_pool(name="nrm", bufs=2))
        fin_p = ctx.enter_context(tc.tile_pool(name="fin", bufs=3 if deep else 2))

        if variant == "causal":
            em_sb = em_p.tile([128, len(CAUSAL_SCHED) * 128], BF16, tag="em", name="emt")
            nc.sync.dma_start(em_sb[:], em[:])
        elif variant == "general":
            em_sb = [em_p.tile([128, TQ], BF16, tag=f"em{i}", name=f"emt{i}") for i in range(NTC)]
            for c in range(NTC):
                nc.sync.dma_start(em_sb[c][:], em[c * 128 : (c + 1) * 128, :])
        wo_sb = [wo_p.tile([128, DIM], BF16, tag=f"wo{i}", name=f"wot{i}") for i in range(NDC)]
        for i in range(NDC):
            nc.sync.dma_start(wo_sb[i][:], woT[i * 128 : (i + 1) * 128, :])
        # normalized attention output, transposed: [head-dim a, tq]
        otn_sb = [otn_p.tile([128, TQ], BF16, tag=f"otn{i}", name=f"otn{i}") for i in range(NDC)]
        otn1_sb = [otn_p.tile([64, TQ], BF16, tag=f"otn1{i}", name=f"otn1{i}") for i in range(NDC)]

        # ---- phase 2: attention + output projection ------------------------
        if variant == "causal":
            sched_idx = {key: i for i, key in enumerate(CAUSAL_SCHED)}

        for a in (0, 1):
            nA = 8 * a + 8 if variant == "causal" else NTC
            asl = slice(a * 512, (a + 1) * 512)
            for hp in range(NDC):
                o_pair = psO.tile([128, 1024], F32, tag="opair", name="o_pair")
                for c in range(nA):
                    csl = slice(c * 128, (c + 1) * 128)
                    # masked-prefix block count within this 512-col group
                    M = max(0, (c - 8 * a) // 2) if variant == "causal" else 0
                    nact = 512 - M * 128
                    s_ps = psS.tile([128, 1024], F32, tag="s", name="s_ps")
                    nc.tensor.matmul(
                        s_ps[:, M * 128 : 512],
                        kt_sb[hp][0:64, csl],
                        qt_sb[hp][0:64, a * 512 + M * 128 : (a + 1) * 512],
                        start=True,
                        stop=True,
                    )
                    nc.tensor.matmul(
                        s_ps[:, 512 + M * 128 : 1024],
                        kt_sb[hp][64:128, csl],
                        qt_sb[hp][64:128, a * 512 + M * 128 : (a + 1) * 512],
                        start=True,
                        stop=True,
                        tile_position=(64, 0),
                    )
                    p_pair = p_p.tile([128, 1024], BF16, tag="p", name="p_pair")
                    s_act = s_ps[:].rearrange("p (h x) -> p h x", h=2)[
                        :, :, M * 128 : 512
                    ]
                    p_act = p_pair[:].rearrange("p (h x) -> p h x", h=2)[
                        :, :, M * 128 : 512
                    ]
                    nc.scalar.activation(p_act, s_act, Exp)
                    if variant == "causal":
                        bb = (c - 8 * a) // 2
                        if 0 <= bb < 4:
                            i = sched_idx[(a, c, bb)]
                            esl = slice(i * 128, (i + 1) * 128)
                            for half in (0, 1):
                                psl = slice(
                                    half * 512 + bb * 128,
                                    half * 512 + bb * 128 + 128,
                                )
                                nc.vector.tensor_mul(
                                    p_pair[:, psl], p_pair[:, psl], em_sb[:, esl]
                                )
                    elif variant == "general":
                        for half in (0, 1):
                            psl = slice(half * 512, half * 512 + 512)
                            nc.vector.tensor_mul(
                                p_pair[:, psl], p_pair[:, psl], em_sb[c][:, asl]
                            )
                    va4 = va_sb[c][:].rearrange("p (q t x) -> p q t x", q=8, t=2)
                    nc.tensor.matmul(
                        o_pair[0:65, M * 128 : 512],
                        va4[:, hp, 0, :],
                        p_pair[:, M * 128 : 512],
                        start=(c == 0),
                        stop=(c == nA - 1),
                    )
                    nc.tensor.matmul(
                        o_pair[0:65, 512 + M * 128 : 1024],
                        va4[:, hp, 1, :],
                        p_pair[:, 512 + M * 128 : 1024],
                        start=(c == 0),
                        stop=(c == nA - 1),
                    )
                # evacuate PSUM early so the next head pair can accumulate
                o_sb = osb_p.tile([128, 1024], F32, tag="osb", name="o_sb")
                nc.vector.tensor_copy(o_sb[:], o_pair[:])
                # normalize: denominators live in row 64 (h0) / row 0 (h1 is
                # [one|V] so its denom is row 0)  -- both heads' AV used
                # lhsT with the ones column, h0 at index 64, h1 at index 0.
                rec = nrm_p.tile([1, 1024], F32, tag="rec", name="rec")
                nc.vector.reciprocal(rec[0:1, 0:1024], o_sb[64:65, 0:1024])
                rec2a = nrm_p.tile([1, 512], F32, tag="rec2a", name="rec2a")
                rec2b = nrm_p.tile([1, 512], F32, tag="rec2b", name="rec2b")
                nc.sync.dma_start(rec2a[:], rec[0:1, 0:512])
                nc.sync.dma_start(rec2b[:], rec[0:1, 512:1024])
                b0 = psB.tile([64, 512], F32, tag="b", name="b0")
                b1 = psB.tile([64, 512], F32, tag="b", name="b1")
                nc.tensor.matmul(
                    b0[:],
                    ones_sb[:].bitcast(mybir.dt.float32r),
                    rec2a[:].bitcast(mybir.dt.float32r),
                    start=True,
                    stop=True,
                )
                nc.tensor.matmul(
                    b1[:],
                    ones_sb[:].bitcast(mybir.dt.float32r),
                    rec2b[:].bitcast(mybir.dt.float32r),
                    start=True,
                    stop=True,
                )
                nc.vector.tensor_tensor(
                    otn_sb[hp][0:64, asl], o_sb[0:64, 0:512], b0[:], mult
                )
                nc.vector.tensor_tensor(
                    otn1_sb[hp][0:64, asl], o_sb[0:64, 512:1024], b1[:], mult
                )
                # move odd head rows to partitions 64..127 for the K=128 wo mm
                nc.sync.dma_start(otn_sb[hp][64:128, asl], otn1_sb[hp][0:64, asl])

            # output projection for this query group
            for tt in range(4 * a, 4 * a + 4):
                tsl = slice((tt % 4) * 128 + a * 512, (tt % 4) * 128 + a * 512 + 128)
                fin = fin_p.tile([128, 1024], F32, tag="fin", name="fin")
                for n in range(2):
                    nsl = slice(n * 512, (n + 1) * 512)
                    f_ps = psF.tile([128, 512], F32, tag="fwo", name="f_ps")
                    for hp in range(NDC):
                        nc.tensor.matmul(
                            f_ps[:],
                            otn_sb[hp][:, tsl],
                            wo_sb[hp][:, nsl],
                            start=(hp == 0),
                            stop=(hp == NDC - 1),
                        )
                    nc.vector.tensor_copy(fin[:, nsl], f_ps[:])
                nc.sync.dma_start(out[tt * 128 : (tt + 1) * 128, :], fin[:])

        ctx.__exit__(None, None, None)

    nc.compile()
    return nc


_NC_CACHE = {}


def _get_nc(variant):
    if variant not in _NC_CACHE:
        _NC_CACHE[variant] = _build(variant)
    return _NC_CACHE[variant]


def kernel(x, mask, wq, wk, wv, wo):
    x = np.asarray(x, dtype=np.float32)
    mask = np.asarray(mask, dtype=np.float32)
    wq = np.asarray(wq, dtype=np.float32)
    wk = np.asarray(wk, dtype=np.float32)
    wv = np.asarray(wv, dtype=np.float32)
    wo = np.asarray(wo, dtype=np.float32)

    m2 = mask[0, 0]  # [T, T]
    emask = np.exp(np.minimum(m2, 60.0)).astype(np.float32)
    tril = np.tril(np.ones((T, T), dtype=np.float32))
    if np.array_equal(emask, tril):
        variant = "causal"
    elif np.all(m2 == 0.0):
        variant = "nomask"
    else:
        variant = "general"

    bf = ml_dtypes.bfloat16
    scale = 1.0 / np.sqrt(HD)
    wqT = np.ascontiguousarray((wq * scale).T).astype(bf)
    wkT = np.ascontiguousarray(wk.T).astype(bf)
    wvT = np.ascontiguousarray(wv.T).astype(bf)
    woT = np.ascontiguousarray(wo.T).astype(bf)

    # per-parity query permutation: parity p takes 128-row blocks p, p+2, ...
    perms = {}
    for p in (0, 1):
        perms[p] = np.concatenate(
            [np.arange(128) + 128 * j for j in range(p, 16, 2)]
        )

    in_maps = []
    for i in range(8):
        b, p = i // 2, i % 2
        perm = perms[p]
        xb = x[b]  # [T, DIM]
        im = {
            "xT": np.ascontiguousarray(xb.T).astype(bf),
            "xqT": np.ascontiguousarray(xb[perm].T).astype(bf),
            "wqT": wqT,
            "wkT": wkT,
            "wvT": wvT,
            "woT": woT,
        }
        if variant == "causal":
            emT = emask.T[:, perm]  # [T(k), TQ] in permuted q space
            slices = []
            for (a, c, bb) in CAUSAL_SCHED:
                col = (4 * a + bb) * 128
                slices.append(emT[c * 128 : (c + 1) * 128, col : col + 128])
            im["em"] = np.ascontiguousarray(np.concatenate(slices, axis=1)).astype(bf)
        elif variant == "general":
            im["em"] = np.ascontiguousarray(emask.T[:, perm]).astype(bf)
        in_maps.append(im)

    nc = _get_nc(variant)
    res = run_bass_kernel_spmd(nc, in_maps, core_ids=list(range(8)))

    out_full = np.empty((B, T, DIM), dtype=np.float32)
    for i in range(8):
        b, p = i // 2, i % 2
        out_full[b, perms[p]] = res.results[i]["out"]
    return out_full


# revision 25
# speedup vs baseline: 1.0165x; 1.0165x over previous
"""Multi-head attention kernel for 8 Trainium2 NeuronCores.

Problem: B=4, T=2048, DIM=1024, 16 heads, head_dim=64, additive mask.
  q,k,v = x@W{q,k,v}.T ; attn = softmax(q k^T/8 + mask) ; out = (attn v)@Wo.T

Sharding (no collectives): core i handles batch b=i//2 and half the queries
(1024 rows, interleaved 128-row blocks by parity p=i%2 so the causal work
profile is identical on every core -> one SPMD graph). K/V projections are
computed for the full 2048 keys on both cores of a pair (duplicated compute,
far cheaper than any 2-rank collective on this chip).

On-chip math: exp(s+m) = exp(s)*exp(m); host precomputes exp(mask).
Softmax denominator comes free from a ones-column appended to V.
All matmul operands bf16; PSUM accumulation and softmax math fp32.
"""

import sys
import numpy as np

sys.path.insert(0, "/opt/trn_rl_repo")

import ml_dtypes  # noqa: E402
from contextlib import ExitStack  # noqa: E402
from concourse import bass, bacc, tile  # noqa: E402
from concourse.bass_utils import run_bass_kernel_spmd  # noqa: E402

mybir = bass.mybir

B, T, DIM, H, HD = 4, 2048, 1024, 16, 64
TQ = 1024          # queries per core
NDC = DIM // 128   # 8 contraction chunks
NTC = T // 128     # 16 key chunks
BF16 = mybir.dt.bfloat16
F32 = mybir.dt.float32


def _causal_sched():
    """Emask-mult schedule for the causal variant.

    Group a in {0,1} covers logical query blocks 4a..4a+3 (columns a*512..).
    Active key chunks for group a: 0..8a+7.  Logical block g=4a+bb holds
    queries whose diagonal chunk is 2g+p (parity p in {0,1}); chunks past
    2g+1 are fully masked for both parities and are skipped entirely (the
    masked blocks form a contiguous prefix of each 512-col group).  Exactly
    one block per chunk needs the emask multiply: bb = (c-8a)//2.
    Returns list of (a, c, bb) in canonical order.
    """
    sched = []
    for a in (0, 1):
        for c in range(8 * a + 8):
            sched.append((a, c, (c - 8 * a) // 2))
    return sched


CAUSAL_SCHED = _causal_sched()


def _build(variant, reps=1):
    """variant: 'causal' | 'general' | 'nomask'."""
    nc = bacc.Bacc("TRN2", target_bir_lowering=False, debug=False, num_devices=8)

    xT = nc.dram_tensor("xT", [DIM, T], BF16, kind="ExternalInput").ap()
    xqT = nc.dram_tensor("xqT", [DIM, TQ], BF16, kind="ExternalInput").ap()
    wqT = nc.dram_tensor("wqT", [DIM, DIM], BF16, kind="ExternalInput").ap()
    wkT = nc.dram_tensor("wkT", [DIM, DIM], BF16, kind="ExternalInput").ap()
    wvT = nc.dram_tensor("wvT", [DIM, DIM], BF16, kind="ExternalInput").ap()
    woT = nc.dram_tensor("woT", [DIM, DIM], BF16, kind="ExternalInput").ap()
    if variant == "causal":
        em = nc.dram_tensor(
            "em", [128, len(CAUSAL_SCHED) * 128], BF16, kind="ExternalInput"
        ).ap()
    elif variant == "general":
        em = nc.dram_tensor("em", [T, TQ], BF16, kind="ExternalInput").ap()
    else:
        em = None
    out = nc.dram_tensor("out", [TQ, DIM], F32, kind="ExternalOutput").ap()

    with tile.TileContext(nc) as tc:
      for _rep in range(reps):
        ctx = ExitStack()
        ctx.__enter__()
        Exp = mybir.ActivationFunctionType.Exp
        mult = mybir.AluOpType.mult

        # ---- resident SBUF pools -------------------------------------------
        qt_p = ctx.enter_context(tc.tile_pool(name="qt", bufs=1))
        kt_p = ctx.enter_context(tc.tile_pool(name="kt", bufs=1))
        va_p = ctx.enter_context(tc.tile_pool(name="va", bufs=1))
        misc_p = ctx.enter_context(tc.tile_pool(name="misc", bufs=1))
        # PSUM pools: psS 2x[128,1024] (4 banks), psO 1x[128,1024] (2 banks),
        # psB 1x[64,512] (1 bank), psF 1x[128,512] (1 bank) = 8 banks.
        psS = ctx.enter_context(tc.tile_pool(name="psS", bufs=2, space="PSUM"))
        psO = ctx.enter_context(tc.tile_pool(name="psO", bufs=1, space="PSUM"))
        psB = ctx.enter_context(tc.tile_pool(name="psB", bufs=1, space="PSUM"))
        psF = ctx.enter_context(tc.tile_pool(name="psF", bufs=1, space="PSUM"))

        qt_sb = [qt_p.tile([128, TQ], BF16, tag=f"qt{i}", name=f"qt{i}") for i in range(NDC)]
        kt_sb = [kt_p.tile([128, T], BF16, tag=f"kt{i}", name=f"kt{i}") for i in range(NDC)]
        # V interleaved with ones column: per key chunk [128, 16*65]
        va_sb = [va_p.tile([128, H * 65], BF16, tag=f"va{i}", name=f"va{i}") for i in range(NTC)]
        # ones for the reciprocal broadcast matmul (row 64: K=1 lhsT whose
        # base partition matches the PSUM row the denominators land in)
        ones_sb = misc_p.tile([128, 64], F32, tag="ones", name="ones")
        nc.vector.memset(ones_sb[64:65, :], 1.0)

        # ---- phase 1: projections ------------------------------------------
        with tc.tile_pool(name="xin", bufs=1) as x_p, tc.tile_pool(
            name="win", bufs=1
        ) as w_p:
            xt_sb = [x_p.tile([128, T], BF16, tag=f"xt{i}", name=f"xt{i}") for i in range(NDC)]
            xq_sb = [x_p.tile([128, TQ], BF16, tag=f"xq{i}", name=f"xq{i}") for i in range(NDC)]
            wq_sb = [w_p.tile([128, DIM], BF16, tag=f"wq{i}", name=f"wq{i}") for i in range(NDC)]
            wk_sb = [w_p.tile([128, DIM], BF16, tag=f"wk{i}", name=f"wk{i}") for i in range(NDC)]
            wv_sb = [w_p.tile([128, DIM], BF16, tag=f"wv{i}", name=f"wv{i}") for i in range(NDC)]
            # issue order matches consumption: V-proj inputs first
            for i in range(NDC):
                s = slice(i * 128, (i + 1) * 128)
                nc.sync.dma_start(wv_sb[i][:], wvT[s, :])
                nc.sync.dma_start(xt_sb[i][:], xT[s, :])
            for i in range(NDC):
                s = slice(i * 128, (i + 1) * 128)
                nc.sync.dma_start(wq_sb[i][:], wqT[s, :])
                nc.sync.dma_start(xq_sb[i][:], xqT[s, :])
            for i in range(NDC):
                s = slice(i * 128, (i + 1) * 128)
                nc.sync.dma_start(wk_sb[i][:], wkT[s, :])

            # ones column of V at position 64 for every head
            for c in range(NTC):
                v4 = va_sb[c][:].rearrange("p (q t x) -> p q t x", q=8, t=2)
                nc.vector.memset(v4[:, :, 0:2, 64:65], 1.0)

            # V projection: V[tk,o] = sum_d xT[d,tk] * wvT[d,o]
            for c in range(NTC):
                csl = slice(c * 128, (c + 1) * 128)
                ps = psS.tile([128, 1024], F32, tag="s", name="v_ps")
                for n in range(2):
                    nsl = slice(n * 512, (n + 1) * 512)
                    for dc in range(NDC):
                        nc.tensor.matmul(
                            ps[:, nsl],
                            xt_sb[dc][:, csl],
                            wv_sb[dc][:, nsl],
                            start=(dc == 0),
                            stop=(dc == NDC - 1),
                        )
                # scatter 16 heads into the 65-stride layout
                v4 = va_sb[c][:].rearrange("p (q t x) -> p q t x", q=8, t=2)
                s4 = ps[:].rearrange("p (q t x) -> p q t x", q=8, t=2)
                nc.vector.tensor_copy(v4[:, :, 0:2, 0:64], s4[:, :, 0:2, :])

            # Q/K projections: out QT[o,t] = sum_d wqT[d,o] * xqT[d,t]
            for oc in range(NDC):
                osl = slice(oc * 128, (oc + 1) * 128)
                ps = psS.tile([128, 1024], F32, tag="s", name="q_ps")
                for n in range(2):
                    nsl = slice(n * 512, (n + 1) * 512)
                    for dc in range(NDC):
                        nc.tensor.matmul(
                            ps[:, nsl],
                            wq_sb[dc][:, osl],
                            xq_sb[dc][:, nsl],
                            start=(dc == 0),
                            stop=(dc == NDC - 1),
                        )
                nc.scalar.copy(qt_sb[oc][:], ps[:])
                for m in range(2):
                    ps = psS.tile([128, 1024], F32, tag="s", name="k_ps")
                    for n in range(2):
                        nsl = slice((2 * m + n) * 512, (2 * m + n + 1) * 512)
                        psl = slice(n * 512, (n + 1) * 512)
                        for dc in range(NDC):
                            nc.tensor.matmul(
                                ps[:, psl],
                                wk_sb[dc][:, osl],
                                xt_sb[dc][:, nsl],
                                start=(dc == 0),
                                stop=(dc == NDC - 1),
                            )
                    nc.scalar.copy(
                        kt_sb[oc][:, m * 1024 : (m + 1) * 1024], ps[:]
                    )


        # ---- phase 2 pools (reuse the projection input space) --------------
        em_p = ctx.enter_context(tc.tile_pool(name="em", bufs=1))
        wo_p = ctx.enter_context(tc.tile_pool(name="wo", bufs=1))
        otn_p = ctx.enter_context(tc.tile_pool(name="otn", bufs=1))
        deep = variant == "causal"
        p_p = ctx.enter_context(tc.tile_pool(name="pp", bufs=6 if deep else 4))
        osb_p = ctx.enter_context(tc.tile_pool(name="osb", bufs=3 if deep else 2))
        nrm_p = ctx.enter_context(tc.tile_pool(name="nrm", bufs=2))
        fin_p = ctx.enter_context(tc.tile_pool(name="fin", bufs=3 if deep else 2))

        if variant == "causal":
            em_sb = em_p.tile([128, len(CAUSAL_SCHED) * 128], BF16, tag="em", name="emt")
            nc.sync.dma_start(em_sb[:], em[:])
        elif variant == "general":
            em_sb = [em_p.tile([128, TQ], BF16, tag=f"em{i}", name=f"emt{i}") for i in range(NTC)]
            for c in range(NTC):
                nc.sync.dma_start(em_sb[c][:], em[c * 128 : (c + 1) * 128, :])
        wo_sb = [wo_p.tile([128, DIM], BF16, tag=f"wo{i}", name=f"wot{i}") for i in range(NDC)]
        for i in range(NDC):
            nc.sync.dma_start(wo_sb[i][:], woT[i * 128 : (i + 1) * 128, :])
        # normalized attention output, transposed: [head-dim a, tq]
        otn_sb = [otn_p.tile([128, TQ], BF16, tag=f"otn{i}", name=f"otn{i}") for i in range(NDC)]
        otn1_sb = [otn_p.tile([64, TQ], BF16, tag=f"otn1{i}", name=f"otn1{i}") for i in range(NDC)]

        # ---- phase 2: attention + output projection ------------------------
        if variant == "causal":
            sched_idx = {key: i for i, key in enumerate(CAUSAL_SCHED)}

        for a in (0, 1):
            nA = 8 * a + 8 if variant == "causal" else NTC
            asl = slice(a * 512, (a + 1) * 512)
            for hp in range(NDC):
                o_pair = psO.tile([128, 1024], F32, tag="opair", name="o_pair")
                for c in range(nA):
                    csl = slice(c * 128, (c + 1) * 128)
                    # masked-prefix block count within this 512-col group
                    M = max(0, (c - 8 * a) // 2) if variant == "causal" else 0
                    nact = 512 - M * 128
                    s_ps = psS.tile([128, 1024], F32, tag="s", name="s_ps")
                    nc.tensor.matmul(
                        s_ps[:, M * 128 : 512],
                        kt_sb[hp][0:64, csl],
                        qt_sb[hp][0:64, a * 512 + M * 128 : (a + 1) * 512],
                        start=True,
                        stop=True,
                    )
                    nc.tensor.matmul(
                        s_ps[:, 512 + M * 128 : 1024],
                        kt_sb[hp][64:128, csl],
                        qt_sb[hp][64:128, a * 512 + M * 128 : (a + 1) * 512],
                        start=True,
                        stop=True,
                        tile_position=(64, 0),
                    )
                    p_pair = p_p.tile([128, 1024], BF16, tag="p", name="p_pair")
                    s_act = s_ps[:].rearrange("p (h x) -> p h x", h=2)[
                        :, :, M * 128 : 512
                    ]
                    p_act = p_pair[:].rearrange("p (h x) -> p h x", h=2)[
                        :, :, M * 128 : 512
                    ]
                    nc.scalar.activation(p_act, s_act, Exp)
                    if variant == "causal":
                        bb = (c - 8 * a) // 2
                        if 0 <= bb < 4:
                            i = sched_idx[(a, c, bb)]
                            esl = slice(i * 128, (i + 1) * 128)
                            for half in (0, 1):
                                psl = slice(
                                    half * 512 + bb * 128,
                                    half * 512 + bb * 128 + 128,
                                )
                                nc.vector.tensor_mul(
                                    p_pair[:, psl], p_pair[:, psl], em_sb[:, esl]
                                )
                    elif variant == "general":
                        for half in (0, 1):
                            psl = slice(half * 512, half * 512 + 512)
                            nc.vector.tensor_mul(
                                p_pair[:, psl], p_pair[:, psl], em_sb[c][:, asl]
                            )
                    va4 = va_sb[c][:].rearrange("p (q t x) -> p q t x", q=8, t=2)
                    nc.tensor.matmul(
                        o_pair[0:65, M * 128 : 512],
                        va4[:, hp, 0, :],
                        p_pair[:, M * 128 : 512],
                        start=(c == 0),
                        stop=(c == nA - 1),
                    )
                    nc.tensor.matmul(
                        o_pair[0:65, 512 + M * 128 : 1024],
                        va4[:, hp, 1, :],
                        p_pair[:, 512 + M * 128 : 1024],
                        start=(c == 0),
                        stop=(c == nA - 1),
                    )
                # evacuate PSUM early so the next head pair can accumulate
                o_sb = osb_p.tile([128, 1024], F32, tag="osb", name="o_sb")
                nc.vector.tensor_copy(o_sb[:], o_pair[:])
                # normalize: denominators live in row 64 (h0) / row 0 (h1 is
                # [one|V] so its denom is row 0)  -- both heads' AV used
                # lhsT with the ones column, h0 at index 64, h1 at index 0.
                rec = nrm_p.tile([128, 1024], mybir.dt.float32r, tag="rec", name="rec")
                with nc.allow_low_precision(reason="f32r recip for denom broadcast"):
                    nc.vector.reciprocal(rec[64:65, 0:1024], o_sb[64:65, 0:1024])
                b0 = psB.tile([64, 512], F32, tag="b", name="b0")
                b1 = psB.tile([64, 512], F32, tag="b", name="b1")
                nc.tensor.matmul(
                    b0[:],
                    ones_sb[64:65, :].bitcast(mybir.dt.float32r),
                    rec[64:65, 0:512],
                    start=True,
                    stop=True,
                    tile_position=(64, 0),
                )
                nc.tensor.matmul(
                    b1[:],
                    ones_sb[64:65, :].bitcast(mybir.dt.float32r),
                    rec[64:65, 512:1024],
                    start=True,
                    stop=True,
                    tile_position=(64, 0),
                )
                nc.vector.tensor_tensor(
                    otn_sb[hp][0:64, asl], o_sb[0:64, 0:512], b0[:], mult
                )
                nc.vector.tensor_tensor(
                    otn1_sb[hp][0:64, asl], o_sb[0:64, 512:1024], b1[:], mult
                )
                # move odd head rows to partitions 64..127 for the K=128 wo mm
                nc.sync.dma_start(otn_sb[hp][64:128, asl], otn1_sb[hp][0:64, asl])

            # output projection for this query group
            for tt in range(4 * a, 4 * a + 4):
                tsl = slice((tt % 4) * 128 + a * 512, (tt % 4) * 128 + a * 512 + 128)
                fin = fin_p.tile([128, 1024], F32, tag="fin", name="fin")
                for n in range(2):
                    nsl = slice(n * 512, (n + 1) * 512)
                    f_ps = psF.tile([128, 512], F32, tag="fwo", name="f_ps")
                    for hp in range(NDC):
                        nc.tensor.matmul(
                            f_ps[:],
                            otn_sb[hp][:, tsl],
                            wo_sb[hp][:, nsl],
                            start=(hp == 0),
                            stop=(hp == NDC - 1),
                        )
                    nc.vector.tensor_copy(fin[:, nsl], f_ps[:])
                nc.sync.dma_start(out[tt * 128 : (tt + 1) * 128, :], fin[:])

        ctx.__exit__(None, None, None)

    nc.compile()
    return nc


_NC_CACHE = {}


def _get_nc(variant):
    if variant not in _NC_CACHE:
        _NC_CACHE[variant] = _build(variant)
    return _NC_CACHE[variant]


def kernel(x, mask, wq, wk, wv, wo):
    x = np.asarray(x, dtype=np.float32)
    mask = np.asarray(mask, dtype=np.float32)
    wq = np.asarray(wq, dtype=np.float32)
    wk = np.asarray(wk, dtype=np.float32)
    wv = np.asarray(wv, dtype=np.float32)
    wo = np.asarray(wo, dtype=np.float32)

    m2 = mask[0, 0]  # [T, T]
    emask = np.exp(np.minimum(m2, 60.0)).astype(np.float32)
    tril = np.tril(np.ones((T, T), dtype=np.float32))
    if np.array_equal(emask, tril):
        variant = "causal"
    elif np.all(m2 == 0.0):
        variant = "nomask"
    else:
        variant = "general"

    bf = ml_dtypes.bfloat16
    scale = 1.0 / np.sqrt(HD)
    wqT = np.ascontiguousarray((wq * scale).T).astype(bf)
    wkT = np.ascontiguousarray(wk.T).astype(bf)
    wvT = np.ascontiguousarray(wv.T).astype(bf)
    woT = np.ascontiguousarray(wo.T).astype(bf)

    # per-parity query permutation: parity p takes 128-row blocks p, p+2, ...
    perms = {}
    for p in (0, 1):
        perms[p] = np.concatenate(
            [np.arange(128) + 128 * j for j in range(p, 16, 2)]
        )

    in_maps = []
    for i in range(8):
        b, p = i // 2, i % 2
        perm = perms[p]
        xb = x[b]  # [T, DIM]
        im = {
            "xT": np.ascontiguousarray(xb.T).astype(bf),
            "xqT": np.ascontiguousarray(xb[perm].T).astype(bf),
            "wqT": wqT,
            "wkT": wkT,
            "wvT": wvT,
            "woT": woT,
        }
        if variant == "causal":
            emT = emask.T[:, perm]  # [T(k), TQ] in permuted q space
            slices = []
            for (a, c, bb) in CAUSAL_SCHED:
                col = (4 * a + bb) * 128
                slices.append(emT[c * 128 : (c + 1) * 128, col : col + 128])
            im["em"] = np.ascontiguousarray(np.concatenate(slices, axis=1)).astype(bf)
        elif variant == "general":
            im["em"] = np.ascontiguousarray(emask.T[:, perm]).astype(bf)
        in_maps.append(im)

    nc = _get_nc(variant)
    res = run_bass_kernel_spmd(nc, in_maps, core_ids=list(range(8)))

    out_full = np.empty((B, T, DIM), dtype=np.float32)
    for i in range(8):
        b, p = i // 2, i % 2
        out_full[b, perms[p]] = res.results[i]["out"]
    return out_full
